# revision 1
# baseline (speedup 1.0000x reference)
"""Dual-stream attention kernel for TRN2 — one batch element per core (v2).

Per-core computation (batch element b):
  qb^T = Wq @ q_b^T          [C, N]   fp16, transposed layout (c on partitions)
  kb^T = Wk @ k_b^T          [C, N]   fp16
  vcomb[tb][tok, h, 0:64]   = (v_b @ Wv^T)    per-head slices   (natural layout)
  vcomb[tb][tok, h, 64:128] = (v_img_b @ Wvim^T)
  per head pair ct (2 heads = one 128-partition q/k tile):
    S^T = kh @ qh^T       K=64 matmuls, row-paired on the PE (lower/upper head)
    E = exp(S^T * scale)  fp16, no max subtraction (logits ~N(0, 0.31))
    U = [vh | vih]^T @ E  M=128: rows 0:64 x-stream, 64:128 img-stream
    r = ones^T @ E        M=1 matmuls, col-paired (strip 0 / strip 32)
    O = U * (1/r)         partition-broadcast recip; DMA partition-shifts for
                          the two misaligned halves (i_lo, x_up)
  x    = merge(O_x)  @ Wp^T  + bp
  x_im = merge(O_im) @ Wpi^T + bpi

All matmul operands are fp16 (10-bit mantissa, ~5e-4 component error).
PSUM accumulation is fp32 throughout.

build_module(loop_n=N) wraps the body in a hardware For_i loop for wall-clock
timing (amortizes the ~60 ms axon dispatch overhead); timing is
data-independent.
"""

import numpy as np
import concourse.bass as bass
import concourse.tile as tile
from concourse import bacc, mybir

P = 128
NTOK = 1024
C = 768
H = 12
DH = 64
CT = C // P  # 6 c-tiles
TB = NTOK // P  # 8 token blocks
QH = 2  # qt halves
KB = 8  # kt blocks
NQ = 512
SCALE = DH**-0.5
F32 = mybir.dt.float32
F16 = mybir.dt.float16
EXP = mybir.ActivationFunctionType.Exp
MULT = mybir.AluOpType.mult
ADD = mybir.AluOpType.add

XNAMES = ("xq", "xk", "xv", "xvi")
WNAMES = ("wq", "wk", "wv", "wvi", "wp", "wpi")


def build_module(num_devices=8, loop_n=1, stages="123"):
    nc = bacc.Bacc(
        "TRN2", target_bir_lowering=False, debug=False, num_devices=num_devices
    )
    d = {}
    for nm in XNAMES:
        d[nm] = nc.dram_tensor(nm, [C, NTOK], F16, kind="ExternalInput").ap()
    for nm in WNAMES:
        d[nm] = nc.dram_tensor(nm, [C, C], F16, kind="ExternalInput").ap()
    d["ones"] = nc.dram_tensor("ones", [P, P], F16, kind="ExternalInput").ap()
    d["bp"] = nc.dram_tensor("bp", [P, C], F32, kind="ExternalInput").ap()
    d["bpi"] = nc.dram_tensor("bpi", [P, C], F32, kind="ExternalInput").ap()
    xo = nc.dram_tensor("xo", [NTOK, C], F32, kind="ExternalOutput").ap()
    xio = nc.dram_tensor("xio", [NTOK, C], F32, kind="ExternalOutput").ap()

    with tile.TileContext(nc) as tc:
        with (
            tc.tile_pool(name="persist", bufs=1) as pp,
            tc.tile_pool(name="wstage", bufs=2) as wpool,
            tc.tile_pool(name="xstage", bufs=2) as xpool,
            tc.tile_pool(name="wk", bufs=8) as wk,
            tc.tile_pool(name="nrm", bufs=8) as nrm,
            tc.tile_pool(name="ubp", bufs=24) as ubp,
            tc.tile_pool(name="rbp", bufs=12) as rbp,
            tc.tile_pool(name="tmp", bufs=8) as tmpp,
            tc.tile_pool(name="ps", bufs=8, space="PSUM") as psp,
        ):
            qbt = pp.tile([P, CT, NTOK], F16, tag="qbt")
            kbt = pp.tile([P, CT, NTOK], F16, tag="kbt")
            # [v | vi] per head: lhsT for the combined AV matmul
            vcomb = pp.tile([P, TB, H, P], F16, tag="vcomb")
            axt = pp.tile([P, CT, NTOK], F16, tag="axt")
            ait = pp.tile([P, CT, NTOK], F16, tag="ait")
            onest = pp.tile([P, P], F16, tag="onest")
            bpr = pp.tile([P, C], F32, tag="bpr")
            bpir = pp.tile([P, C], F32, tag="bpir")

            def stage1():
                nc.sync.dma_start(bpr[:], d["bp"])
                nc.sync.dma_start(bpir[:], d["bpi"])
                nc.sync.dma_start(onest[:], d["ones"])

                for src, wsrc, mode in (
                    ("xv", "wv", "nat_v"),
                    ("xvi", "wvi", "nat_vi"),
                    ("xq", "wq", "tr_q"),
                    ("xk", "wk", "tr_k"),
                ):
                    xt = xpool.tile([P, CT, NTOK], F16, tag="xt")
                    nc.sync.dma_start(
                        xt[:], d[src].rearrange("(ct p) n -> p ct n", p=P)
                    )
                    wt = wpool.tile([P, CT, C], F16, tag="wt")
                    nc.sync.dma_start(
                        wt[:], d[wsrc].rearrange("(ct p) c -> p ct c", p=P)
                    )
                    if mode.startswith("tr"):
                        dst = qbt if mode == "tr_q" else kbt
                        for co in range(CT):
                            for nh in range(QH):
                                ps = psp.tile([P, NQ], F32, tag="ps")
                                for ci in range(CT):
                                    nc.tensor.matmul(
                                        ps[:],
                                        wt[:, ci, co * P : (co + 1) * P],
                                        xt[:, ci, nh * NQ : (nh + 1) * NQ],
                                        start=(ci == 0),
                                        stop=(ci == CT - 1),
                                    )
                                nc.vector.tensor_copy(
                                    dst[:, co, nh * NQ : (nh + 1) * NQ], ps[:]
                                )
                    else:
                        off = 0 if mode == "nat_v" else DH
                        for tb in range(TB):
                            for c0, cw in ((0, 512), (512, 256)):
                                ps = psp.tile([P, NQ], F32, tag="ps")
                                for ci in range(CT):
                                    nc.tensor.matmul(
                                        ps[:, :cw],
                                        xt[:, ci, tb * P : (tb + 1) * P],
                                        wt[:, ci, c0 : c0 + cw],
                                        start=(ci == 0),
                                        stop=(ci == CT - 1),
                                    )
                                h0, h1 = c0 // DH, (c0 + cw) // DH
                                nc.vector.tensor_copy(
                                    vcomb[:, tb, h0:h1, off : off + DH],
                                    ps[:, :cw].rearrange("p (h dh) -> p h dh", dh=DH),
                                )

            def stage2():
                stash = []
                for ct in range(CT):
                    h_lo, h_up = 2 * ct, 2 * ct + 1
                    for qh in range(QH):
                        qsl = slice(qh * NQ, (qh + 1) * NQ)
                        u_lo = psp.tile([P, NQ], F32, tag="ps")
                        u_up = psp.tile([P, NQ], F32, tag="ps")
                        r_lo = psp.tile([1, NQ], F32, tag="ps")
                        r_up = psp.tile([33, NQ], F32, tag="ps")
                        # software-pipelined: scores/exp run one kb ahead of
                        # the U/rowsum consumers so the PE never waits on ACT.
                        es = []
                        for kb in range(KB):
                            ksl = slice(kb * P, (kb + 1) * P)
                            s_lo = psp.tile([P, NQ], F32, tag="ps")
                            s_up = psp.tile([P, NQ], F32, tag="ps")
                            nc.tensor.matmul(
                                s_lo[:], kbt[0:DH, ct, ksl], qbt[0:DH, ct, qsl],
                                start=True, stop=True,
                            )
                            nc.tensor.matmul(
                                s_up[:], kbt[DH:P, ct, ksl], qbt[DH:P, ct, qsl],
                                start=True, stop=True,
                            )
                            e_lo = wk.tile([P, NQ], F16, tag="e")
                            e_up = wk.tile([P, NQ], F16, tag="e")
                            nc.scalar.activation(e_lo[:], s_lo[:], EXP, scale=SCALE)
                            nc.scalar.activation(e_up[:], s_up[:], EXP, scale=SCALE)
                            es.append((e_lo, e_up))
                            if kb > 0:
                                pe_lo, pe_up = es[kb - 1]
                                st, sp = kb - 1 == 0, False
                                pkb = kb - 1
                                nc.tensor.matmul(
                                    u_lo[:], vcomb[:, pkb, h_lo, :], pe_lo[:],
                                    start=st, stop=sp,
                                )
                                nc.tensor.matmul(
                                    u_up[:], vcomb[:, pkb, h_up, :], pe_up[:],
                                    start=st, stop=sp,
                                )
                                nc.tensor.matmul(
                                    r_lo[:], onest[:, 0:1], pe_lo[:],
                                    start=st, stop=sp,
                                )
                                nc.tensor.matmul(
                                    r_up[32:33, :], onest[:, 0:1], pe_up[:],
                                    start=st, stop=sp,
                                )
                        pe_lo, pe_up = es[KB - 1]
                        nc.tensor.matmul(
                            u_lo[:], vcomb[:, KB - 1, h_lo, :], pe_lo[:],
                            start=False, stop=True,
                        )
                        nc.tensor.matmul(
                            u_up[:], vcomb[:, KB - 1, h_up, :], pe_up[:],
                            start=False, stop=True,
                        )
                        nc.tensor.matmul(
                            r_lo[:], onest[:, 0:1], pe_lo[:], start=False, stop=True,
                        )
                        nc.tensor.matmul(
                            r_up[32:33, :], onest[:, 0:1], pe_up[:],
                            start=False, stop=True,
                        )

                        # ---- drain PSUM fast (frees banks for the next group) ----
                        ub_lo = ubp.tile([P, NQ], F16, tag="ub")
                        ub_up = ubp.tile([P, NQ], F16, tag="ub")
                        rb = rbp.tile([33, NQ], F16, tag="rb")
                        nc.vector.tensor_copy(ub_lo[:], u_lo[:])
                        nc.vector.tensor_copy(ub_up[:], u_up[:])
                        nc.vector.tensor_copy(rb[0:1, :], r_lo[:])
                        nc.vector.tensor_copy(rb[32:33, :], r_up[32:33, :])
                        stash.append((ct, qsl, ub_lo, ub_up, rb))

                # ---- deferred normalize pass (overlaps stage 3 setup) ----
                for ct, qsl, ub_lo, ub_up, rb in stash:
                    rc_l = nrm.tile([1, NQ], F16, tag="rc")
                    with nc.allow_low_precision(reason="softmax recip in fp16"):
                        nc.vector.reciprocal(rc_l[:], rb[0:1, :])
                    rp_l = psp.tile([P, NQ], F32, tag="ps")
                    nc.tensor.matmul(
                        rp_l[:], onest[0:1, :], rc_l[:], start=True, stop=True
                    )
                    nc.vector.tensor_tensor(
                        axt[0:DH, ct, qsl], ub_lo[0:DH, :], rp_l[0:DH, :], MULT
                    )
                    t_il = tmpp.tile([P, NQ], F16, tag="tshift")
                    nc.vector.tensor_tensor(
                        t_il[DH:P, :], ub_lo[DH:P, :], rp_l[DH:P, :], MULT
                    )
                    nc.sync.dma_start(ait[0:DH, ct, qsl], t_il[DH:P, :])

                    rs_u = nrm.tile([33, NQ], F16, tag="rsu")
                    with nc.allow_low_precision(reason="softmax recip in fp16"):
                        nc.vector.reciprocal(rs_u[32:33, :], rb[32:33, :])
                    rc_u = nrm.tile([1, NQ], F16, tag="rc")
                    nc.sync.dma_start(rc_u[:], rs_u[32:33, :])
                    rp_u = psp.tile([P, NQ], F32, tag="ps")
                    nc.tensor.matmul(
                        rp_u[:], onest[0:1, :], rc_u[:], start=True, stop=True
                    )
                    t_xu = tmpp.tile([P, NQ], F16, tag="tshift")
                    nc.vector.tensor_tensor(
                        t_xu[0:DH, :], ub_up[0:DH, :], rp_u[0:DH, :], MULT
                    )
                    nc.sync.dma_start(axt[DH:P, ct, qsl], t_xu[0:DH, :])
                    nc.vector.tensor_tensor(
                        ait[DH:P, ct, qsl], ub_up[DH:P, :], rp_u[DH:P, :], MULT
                    )

            def stage3():
                for dst_dram, src, w_nm, bias_t in (
                    (xo, axt, "wp", bpr),
                    (xio, ait, "wpi", bpir),
                ):
                    wt = wpool.tile([P, CT, C], F16, tag="wt")
                    nc.sync.dma_start(
                        wt[:], d[w_nm].rearrange("(ct p) c -> p ct c", p=P)
                    )
                    for tb in range(TB):
                        for c0, cw in ((0, 512), (512, 256)):
                            ps = psp.tile([P, NQ], F32, tag="ps")
                            for ci in range(CT):
                                nc.tensor.matmul(
                                    ps[:, :cw],
                                    src[:, ci, tb * P : (tb + 1) * P],
                                    wt[:, ci, c0 : c0 + cw],
                                    start=(ci == 0),
                                    stop=(ci == CT - 1),
                                )
                            ot = wk.tile([P, NQ], F32, tag="ot")
                            nc.vector.tensor_tensor(
                                ot[:, :cw], ps[:, :cw], bias_t[:, c0 : c0 + cw], ADD
                            )
                            nc.sync.dma_start(
                                dst_dram[tb * P : (tb + 1) * P, c0 : c0 + cw],
                                ot[:, :cw],
                            )

            def body():
                if "1" in stages:
                    stage1()
                if "2" in stages:
                    stage2()
                if "3" in stages:
                    stage3()

            if loop_n == 1:
                body()
            else:
                with tc.For_i(0, loop_n, 1):
                    body()

    nc.compile()
    return nc


def make_in_maps(q, k, v, v_img, Wq, Wk, Wv, Wvim, Wp, bp, Wpi, bpi, n_cores=8):
    """Host-side prep: per-core transposed fp16 activations + shared fp16 weights."""
    f = np.float32
    h = np.float16
    shared = {
        "wq": np.asarray(Wq, f).T.astype(h),
        "wk": np.asarray(Wk, f).T.astype(h),
        "wv": np.asarray(Wv, f).T.astype(h),
        "wvi": np.asarray(Wvim, f).T.astype(h),
        "wp": np.asarray(Wp, f).T.astype(h),
        "wpi": np.asarray(Wpi, f).T.astype(h),
        "ones": np.ones((P, P), h),
        "bp": np.ascontiguousarray(np.broadcast_to(np.asarray(bp, f), (P, C))),
        "bpi": np.ascontiguousarray(np.broadcast_to(np.asarray(bpi, f), (P, C))),
    }
    q = np.asarray(q, f)
    k = np.asarray(k, f)
    v = np.asarray(v, f)
    vi = np.asarray(v_img, f)
    in_maps = []
    for b in range(n_cores):
        in_maps.append(
            {
                "xq": np.ascontiguousarray(q[:, b, :].T).astype(h),
                "xk": np.ascontiguousarray(k[:, b, :].T).astype(h),
                "xv": np.ascontiguousarray(v[:, b, :].T).astype(h),
                "xvi": np.ascontiguousarray(vi[:, b, :].T).astype(h),
                **shared,
            }
        )
    return in_maps


# ---------------------------------------------------------------------------
# Harness entry point: full inputs in, full outputs out.
# Shards batch B=8 across the 8 NeuronCores (data parallel), no collectives.
# ---------------------------------------------------------------------------

_NC_CACHE = {}


def _get_module():
    if "nc" not in _NC_CACHE:
        _NC_CACHE["nc"] = build_module(num_devices=8)
    return _NC_CACHE["nc"]


def kernel(q, k, v, v_img, Wq, Wk, Wv, Wvim, Wp, bp, Wpi, bpi):
    from concourse.bass_utils import run_bass_kernel_spmd

    B = np.asarray(q).shape[1]
    nc = _get_module()
    in_maps = make_in_maps(q, k, v, v_img, Wq, Wk, Wv, Wvim, Wp, bp, Wpi, bpi,
                           n_cores=B)
    res = run_bass_kernel_spmd(nc, in_maps, core_ids=list(range(B)), trace=False)
    x = np.stack([res.results[b]["xo"] for b in range(B)])
    x_im = np.stack([res.results[b]["xio"] for b in range(B)])
    return (x, x_im)



# revision 9
# speedup vs baseline: 1.2569x; 1.2569x over previous
"""Dual-stream attention kernel for TRN2 — one batch element per core (v3b).

Per-core computation (batch element b):
  qbt = Wq @ q_b^T   [C, N] fp16 transposed layout (chan on partitions)
  kbt = Wk @ k_b^T   [C, N] fp16
  vcomb[tb][tok, h, 0:64|64:128] = v_b @ Wv^T | v_img_b @ Wvim^T
  12 groups (q-half qh x head-pair ct), per group:
    S^T = kh qh^T   (K=64 matmuls, head pair stacked on output partitions)
    E = exp(S^T * scale)  fp16 (no max subtraction; logits ~N(0, 0.31))
    U = [vh|vih]^T E  (K=128, PSUM-accumulated over 8 key blocks)
    rowsum r = 1^T (sum_kb E_kb): the kb-sum runs as a DVE ladder of fp16
    adds (2x mode), so the PE does ONE ones-matmul per half instead of 8.
    normalize: rp = bcast(1/r) via K=1 matmul; U * rp -> axt/ait (with
    partition-shift DMAs for the two misaligned halves)
  x    = merge(O_x)  @ Wp^T  + bp   (fp16 out)
  x_im = merge(O_im) @ Wpi^T + bpi  (fp16 out)

Emission order = engine schedule (each engine executes in order), so the
program is laid out as 12 "windows" (one attention group each) with other
PE work injected between kb steps as fillers:
  upfront: q-proj (first half), k-proj co0-1, v/vi c0-chunk projections
  w0-3:  + k-proj co2-5, v/vi c512 chunks     w4-6: + q-proj second half
  w7-10: + qh0 output-projection blocks       w11+tail: last ob + qh1 obs
The normalize pipeline is depth-2: group i's rowsum matmul + reciprocal
run at the end of window i; the broadcast matmul + multiplies run inside
window i+1 (between kb steps), so no PE instruction waits on a DVE chain.

PSUM (8 banks): 3 score/filler slots + 4 U slots + 1 rowsum/bcast slot.
"""

import numpy as np
import concourse.bass as bass
import concourse.tile as tile
from concourse import bacc, mybir

P = 128
NTOK = 1024
C = 768
H = 12
DH = 64
CT = C // P  # 6 c-tiles
TB = NTOK // P  # 8 token blocks
QH = 2  # q halves
KB = 8  # key blocks
NQ = 512
SCALE = DH**-0.5
F32 = mybir.dt.float32
F16 = mybir.dt.float16
EXP = mybir.ActivationFunctionType.Exp
MULT = mybir.AluOpType.mult
ADD = mybir.AluOpType.add

XNAMES = ("xq", "xk", "xv", "xvi")
WNAMES = ("wq", "wk", "wv", "wvi", "wp", "wpi")


def build_module(num_devices=8, loop_n=1, stages="123"):
    nc = bacc.Bacc(
        "TRN2", target_bir_lowering=False, debug=False, num_devices=num_devices
    )
    d = {}
    for nm in XNAMES:
        d[nm] = nc.dram_tensor(nm, [C, NTOK], F16, kind="ExternalInput").ap()
    for nm in WNAMES:
        d[nm] = nc.dram_tensor(nm, [C, C], F16, kind="ExternalInput").ap()
    d["ones"] = nc.dram_tensor("ones", [P, P], F16, kind="ExternalInput").ap()
    d["bp"] = nc.dram_tensor("bp", [P, C], F16, kind="ExternalInput").ap()
    d["bpi"] = nc.dram_tensor("bpi", [P, C], F16, kind="ExternalInput").ap()
    xo = nc.dram_tensor("xo", [NTOK, C], F16, kind="ExternalOutput").ap()
    xio = nc.dram_tensor("xio", [NTOK, C], F16, kind="ExternalOutput").ap()

    with tile.TileContext(nc) as tc:
        with (
            tc.tile_pool(name="persist", bufs=1) as pp,
            tc.tile_pool(name="wk", bufs=6) as wk,
            tc.tile_pool(name="espool", bufs=4) as espool,
            tc.tile_pool(name="ubp", bufs=4) as ubp,
            tc.tile_pool(name="nrm", bufs=3) as nrm,
            tc.tile_pool(name="tmp", bufs=3) as tmpp,
            tc.tile_pool(name="ot", bufs=3) as otp,
            tc.tile_pool(name="pss", bufs=4, space="PSUM") as pss,
            tc.tile_pool(name="psu", bufs=2, space="PSUM") as psu,
            tc.tile_pool(name="psn", bufs=2, space="PSUM") as psn,
        ):
            qbt = pp.tile([P, CT, NTOK], F16, tag="qbt")
            kbt = pp.tile([P, CT, NTOK], F16, tag="kbt")
            vcomb = pp.tile([P, TB, H, P], F16, tag="vcomb")
            axt = pp.tile([P, CT, NTOK], F16, tag="axt")
            ait = pp.tile([P, CT, NTOK], F16, tag="ait")
            onest = pp.tile([P, P], F16, tag="onest")
            bpr = pp.tile([P, C], F16, tag="bpr")
            bpir = pp.tile([P, C], F16, tag="bpir")
            wpt = pp.tile([P, CT, C], F16, tag="wpt")
            wpit = pp.tile([P, CT, C], F16, tag="wpit")
            xq = pp.tile([P, CT, NTOK], F16, tag="xq")
            xk = pp.tile([P, CT, NTOK], F16, tag="xk")
            xv = pp.tile([P, CT, NTOK], F16, tag="xv")
            xvi = pp.tile([P, CT, NTOK], F16, tag="xvi")
            wq = pp.tile([P, CT, C], F16, tag="wq")
            wk_ = pp.tile([P, CT, C], F16, tag="wk")
            wv_ = pp.tile([P, CT, C], F16, tag="wv")
            wvi_ = pp.tile([P, CT, C], F16, tag="wvi")

            def chunk_dma(dst, nm):
                for ci in range(CT):
                    nc.sync.dma_start(
                        dst[:, ci, :], d[nm][ci * P : (ci + 1) * P, :]
                    )

            def proj_chain_t(dst, xt, wt, co, nh):
                """One output chain of a transposed projection (W @ x^T)."""
                ps = pss.tile([P, NQ], F32, tag="ps")
                for ci in range(CT):
                    nc.tensor.matmul(
                        ps[:],
                        wt[:, ci, co * P : (co + 1) * P],
                        xt[:, ci, nh * NQ : (nh + 1) * NQ],
                        start=(ci == 0),
                        stop=(ci == CT - 1),
                    )
                nc.vector.tensor_copy(dst[:, co, nh * NQ : (nh + 1) * NQ], ps[:])

            def vvi_chain(xt, wt, tb, c0, cw, off):
                """One (token-block, col-chunk) chain of a v/v_img projection."""
                h0, h1 = c0 // DH, (c0 + cw) // DH
                ps = pss.tile([P, NQ], F32, tag="ps")
                for ci in range(CT):
                    nc.tensor.matmul(
                        ps[:, :cw],
                        xt[:, ci, tb * P : (tb + 1) * P],
                        wt[:, ci, c0 : c0 + cw],
                        start=(ci == 0),
                        stop=(ci == CT - 1),
                    )
                nc.vector.tensor_copy(
                    vcomb[:, tb, h0:h1, off : off + DH],
                    ps[:, :cw].rearrange("p (h dh) -> p h dh", dh=DH),
                )

            def out_block(tb, stream):
                """One (token-block, stream) of the output projection."""
                src, wt, bias_t, dst_dram = (
                    (axt, wpt, bpr, xo) if stream == 0 else (ait, wpit, bpir, xio)
                )
                ot = otp.tile([P, C], F16, tag="ot")
                for c0, cw in ((0, 512), (512, 256)):
                    ps = pss.tile([P, NQ], F32, tag="ps")
                    for ci in range(CT):
                        nc.tensor.matmul(
                            ps[:, :cw],
                            src[:, ci, tb * P : (tb + 1) * P],
                            wt[:, ci, c0 : c0 + cw],
                            start=(ci == 0),
                            stop=(ci == CT - 1),
                        )
                    nc.vector.tensor_tensor(
                        ot[:, c0 : c0 + cw], ps[:, :cw],
                        bias_t[:, c0 : c0 + cw], ADD,
                    )
                nc.sync.dma_start(dst_dram[tb * P : (tb + 1) * P, :], ot[:])

            # ---- attention group (window) with injected fillers ----

            def norm_lo(stash):
                ct, qsl, ub_lo, ub_up, rc_l, rs_u = stash
                rp = psn.tile([P, NQ], F32, tag="rn")
                nc.tensor.matmul(
                    rp[:], onest[0:1, :], rc_l[:], start=True, stop=True
                )
                nc.vector.tensor_tensor(
                    axt[0:DH, ct, qsl], ub_lo[0:DH, :], rp[0:DH, :], MULT
                )
                t_il = tmpp.tile([P, NQ], F16, tag="tshift")
                nc.vector.tensor_tensor(
                    t_il[DH:P, :], ub_lo[DH:P, :], rp[DH:P, :], MULT
                )
                nc.sync.dma_start(ait[0:DH, ct, qsl], t_il[DH:P, :])

            def norm_up(stash):
                ct, qsl, ub_lo, ub_up, rc_l, rs_u = stash
                rp = psn.tile([P, NQ], F32, tag="rn")
                nc.tensor.matmul(
                    rp[:], onest[32:33, :], rs_u[32:33, :], start=True, stop=True
                )
                nc.vector.tensor_tensor(
                    ait[DH:P, ct, qsl], ub_up[DH:P, :], rp[DH:P, :], MULT
                )
                t_xu = tmpp.tile([P, NQ], F16, tag="tshift")
                nc.vector.tensor_tensor(
                    t_xu[0:DH, :], ub_up[0:DH, :], rp[0:DH, :], MULT
                )
                nc.sync.dma_start(axt[DH:P, ct, qsl], t_xu[0:DH, :])

            def window(ct, qh, prev, fillers):
                """One attention group. prev's normalize + fillers are
                interleaved between kb steps. Returns this group's stash."""
                h_lo, h_up = 2 * ct, 2 * ct + 1
                qsl = slice(qh * NQ, (qh + 1) * NQ)
                u_lo = psu.tile([P, NQ], F32, tag="u")
                u_up = psu.tile([P, NQ], F32, tag="u")
                es = []
                fq = list(fillers)
                run_l = run_u = None
                for kb in range(KB):
                    ksl = slice(kb * P, (kb + 1) * P)
                    s_lo = pss.tile([P, NQ], F32, tag="ps")
                    s_up = pss.tile([P, NQ], F32, tag="ps")
                    nc.tensor.matmul(
                        s_lo[:], kbt[0:DH, ct, ksl], qbt[0:DH, ct, qsl],
                        start=True, stop=True,
                    )
                    nc.tensor.matmul(
                        s_up[:], kbt[DH:P, ct, ksl], qbt[DH:P, ct, qsl],
                        start=True, stop=True,
                    )
                    e_lo = wk.tile([P, NQ], F16, tag="e")
                    e_up = wk.tile([P, NQ], F16, tag="e")
                    nc.scalar.activation(e_lo[:], s_lo[:], EXP, scale=SCALE)
                    nc.scalar.activation(e_up[:], s_up[:], EXP, scale=SCALE)
                    es.append((e_lo, e_up))
                    if kb > 0:
                        pkb = kb - 1
                        st = pkb == 0
                        nc.tensor.matmul(
                            u_lo[:], vcomb[:, pkb, h_lo, :], es[pkb][0][:],
                            start=st, stop=False,
                        )
                        nc.tensor.matmul(
                            u_up[:], vcomb[:, pkb, h_up, :], es[pkb][1][:],
                            start=st, stop=False,
                        )
                    # esum ladder (DVE, fp16 2x mode)
                    if kb == 1:
                        run_l = espool.tile([P, NQ], F16, tag="run")
                        run_u = espool.tile([P, NQ], F16, tag="run")
                        nc.vector.tensor_tensor(
                            run_l[:], es[0][0][:], es[1][0][:], ADD
                        )
                        nc.gpsimd.tensor_tensor(
                            run_u[:], es[0][1][:], es[1][1][:], ADD
                        )
                    elif kb > 1:
                        nrun_l = espool.tile([P, NQ], F16, tag="run")
                        nrun_u = espool.tile([P, NQ], F16, tag="run")
                        nc.vector.tensor_tensor(
                            nrun_l[:], run_l[:], es[kb][0][:], ADD
                        )
                        eng = nc.vector if kb == KB - 1 else nc.gpsimd
                        eng.tensor_tensor(
                            nrun_u[:], run_u[:], es[kb][1][:], ADD
                        )
                        run_l, run_u = nrun_l, nrun_u
                    # injected work between kb steps
                    if kb == 2 and prev is not None:
                        norm_lo(prev)
                    if kb == 4 and prev is not None:
                        norm_up(prev)
                    if kb in (1, 3) and fq:
                        fq.pop(0)()
                nc.tensor.matmul(
                    u_lo[:], vcomb[:, KB - 1, h_lo, :], es[KB - 1][0][:],
                    start=False, stop=True,
                )
                nc.tensor.matmul(
                    u_up[:], vcomb[:, KB - 1, h_up, :], es[KB - 1][1][:],
                    start=False, stop=True,
                )
                while fq:
                    fq.pop(0)()
                # drain U to SBUF f16 (frees PSUM; DVE multiplies may only
                # read one PSUM operand)
                ub_lo = ubp.tile([P, NQ], F16, tag="ub")
                ub_up = ubp.tile([P, NQ], F16, tag="ub")
                nc.vector.tensor_copy(ub_lo[:], u_lo[:])
                nc.vector.tensor_copy(ub_up[:], u_up[:])
                # rowsum (single ones-matmul per half) + reciprocal
                r2 = psn.tile([33, NQ], F32, tag="rn")
                nc.tensor.matmul(
                    r2[0:1, :], onest[:, 0:1], run_l[:], start=True, stop=True
                )
                nc.tensor.matmul(
                    r2[32:33, :], onest[:, 0:1], run_u[:], start=True, stop=True
                )
                rc_l = nrm.tile([1, NQ], F16, tag="rc")
                rs_u = nrm.tile([33, NQ], F16, tag="rs")
                with nc.allow_low_precision(reason="softmax recip in fp16"):
                    nc.vector.reciprocal(rc_l[:], r2[0:1, :])
                    nc.vector.reciprocal(rs_u[32:33, :], r2[32:33, :])
                return (ct, qsl, ub_lo, ub_up, rc_l, rs_u)

            def body():
                # ---- DMA issuance (in needed-first order) ----
                nc.sync.dma_start(wq[:, 0, 0:P], d["wq"][0:P, 0:P])
                nc.sync.dma_start(xq[:, 0, 0:NQ], d["xq"][0:P, 0:NQ])
                for wt_, wnm, xt_, xnm in (
                    (wq, "wq", xq, "xq"), (wk_, "wk", xk, "xk"),
                    (wv_, "wv", xv, "xv"), (wvi_, "wvi", xvi, "xvi"),
                ):
                    for ci in range(CT):
                        if wt_ is wq and ci == 0:
                            nc.sync.dma_start(
                                wt_[:, 0, P:C], d[wnm][0:P, P:C]
                            )
                            nc.sync.dma_start(
                                xt_[:, 0, NQ:NTOK], d[xnm][0:P, NQ:NTOK]
                            )
                            continue
                        nc.sync.dma_start(
                            wt_[:, ci, :], d[wnm][ci * P : (ci + 1) * P, :]
                        )
                        nc.sync.dma_start(
                            xt_[:, ci, :], d[xnm][ci * P : (ci + 1) * P, :]
                        )
                nc.sync.dma_start(onest[:], d["ones"])
                nc.sync.dma_start(bpr[:], d["bp"])
                nc.sync.dma_start(bpir[:], d["bpi"])
                chunk_dma(wpt, "wp")
                chunk_dma(wpit, "wpi")

                # ---- upfront PE work ----
                for co in range(CT):
                    proj_chain_t(qbt, xq, wq, co, 0)
                for co in range(CT):
                    for nh in range(QH):
                        proj_chain_t(kbt, xk, wk_, co, nh)
                for tb in range(TB):
                    vvi_chain(xv, wv_, tb, 0, 512, 0)
                    vvi_chain(xvi, wvi_, tb, 0, 512, DH)

                # ---- filler schedules per window ----
                def vfill(tb):
                    return [
                        lambda tb=tb: vvi_chain(xv, wv_, tb, 512, 256, 0),
                        lambda tb=tb: vvi_chain(xvi, wvi_, tb, 512, 256, DH),
                    ]

                def qfill(cos):
                    return [
                        lambda co=co: proj_chain_t(qbt, xq, wq, co, 1)
                        for co in cos
                    ]

                def ofill(obs):
                    return [
                        lambda tb=tb, s=s: out_block(tb, s) for tb, s in obs
                    ]

                fillers = {
                    0: vfill(0) + vfill(1),
                    1: vfill(2) + vfill(3),
                    2: vfill(4) + vfill(5),
                    3: vfill(6) + vfill(7),
                    4: qfill((0, 1)),
                    5: qfill((2, 3)),
                    6: qfill((4, 5)),
                    7: ofill(((0, 0), (0, 1))),
                    8: ofill(((1, 0), (1, 1))),
                    9: ofill(((2, 0), (2, 1))),
                    10: [],
                    11: [],
                }

                groups = [(qh, ct) for qh in range(QH) for ct in range(CT)]
                prev = None
                for i, (qh, ct) in enumerate(groups):
                    prev = window(ct, qh, prev, fillers[i])

                # ---- tail: last normalize + remaining output blocks ----
                out_block(3, 0)
                norm_lo(prev)
                out_block(3, 1)
                norm_up(prev)
                for tb in range(4, TB):
                    out_block(tb, 0)
                    out_block(tb, 1)

            if loop_n == 1:
                body()
            else:
                with tc.For_i(0, loop_n, 1):
                    body()

    nc.compile()
    return nc


def make_in_maps(q, k, v, v_img, Wq, Wk, Wv, Wvim, Wp, bp, Wpi, bpi, n_cores=8):
    """Host-side prep: per-core transposed fp16 activations + shared fp16 weights."""
    f = np.float32
    h = np.float16
    shared = {
        "wq": np.asarray(Wq, f).T.astype(h),
        "wk": np.asarray(Wk, f).T.astype(h),
        "wv": np.asarray(Wv, f).T.astype(h),
        "wvi": np.asarray(Wvim, f).T.astype(h),
        "wp": np.asarray(Wp, f).T.astype(h),
        "wpi": np.asarray(Wpi, f).T.astype(h),
        "ones": np.ones((P, P), h),
        "bp": np.ascontiguousarray(np.broadcast_to(np.asarray(bp, f), (P, C))).astype(h),
        "bpi": np.ascontiguousarray(np.broadcast_to(np.asarray(bpi, f), (P, C))).astype(h),
    }
    q = np.asarray(q, f)
    k = np.asarray(k, f)
    v = np.asarray(v, f)
    vi = np.asarray(v_img, f)
    in_maps = []
    for b in range(n_cores):
        in_maps.append(
            {
                "xq": np.ascontiguousarray(q[:, b, :].T).astype(h),
                "xk": np.ascontiguousarray(k[:, b, :].T).astype(h),
                "xv": np.ascontiguousarray(v[:, b, :].T).astype(h),
                "xvi": np.ascontiguousarray(vi[:, b, :].T).astype(h),
                **shared,
            }
        )
    return in_maps


# ---------------------------------------------------------------------------
# Harness entry point: full inputs in, full outputs out.
# Shards batch B=8 across the 8 NeuronCores (data parallel), no collectives.
# ---------------------------------------------------------------------------

_NC_CACHE = {}


def _get_module():
    if "nc" not in _NC_CACHE:
        _NC_CACHE["nc"] = build_module(num_devices=8)
    return _NC_CACHE["nc"]


def kernel(q, k, v, v_img, Wq, Wk, Wv, Wvim, Wp, bp, Wpi, bpi):
    from concourse.bass_utils import run_bass_kernel_spmd

    B = np.asarray(q).shape[1]
    nc = _get_module()
    in_maps = make_in_maps(q, k, v, v_img, Wq, Wk, Wv, Wvim, Wp, bp, Wpi, bpi,
                           n_cores=B)
    res = run_bass_kernel_spmd(nc, in_maps, core_ids=list(range(B)), trace=False)
    x = np.stack([res.results[b]["xo"].astype(np.float32) for b in range(B)])
    x_im = np.stack([res.results[b]["xio"].astype(np.float32) for b in range(B)])
    return (x, x_im)


# revision 15
# speedup vs baseline: 1.5024x; 1.1954x over previous
"""Dual-stream attention kernel for TRN2 — one batch element per core (v3b).

Per-core computation (batch element b):
  qbt = Wq @ q_b^T   [C, N] fp16 transposed layout (chan on partitions)
  kbt = Wk @ k_b^T   [C, N] fp16
  vcomb[tb][tok, h, 0:64|64:128] = v_b @ Wv^T | v_img_b @ Wvim^T
  12 groups (q-half qh x head-pair ct), per group:
    S^T = kh qh^T   (K=64 matmuls, head pair stacked on output partitions)
    E = exp(S^T * scale)  fp16 (no max subtraction; logits ~N(0, 0.31))
    U = [vh|vih]^T E  (K=128, PSUM-accumulated over 8 key blocks)
    rowsum r = 1^T (sum_kb E_kb): the kb-sum runs as a DVE ladder of fp16
    adds (2x mode), so the PE does ONE ones-matmul per half instead of 8.
    normalize: rp = bcast(1/r) via K=1 matmul; U * rp -> axt/ait (with
    partition-shift DMAs for the two misaligned halves)
  x    = merge(O_x)  @ Wp^T  + bp   (fp16 out)
  x_im = merge(O_im) @ Wpi^T + bpi  (fp16 out)

Emission order = engine schedule (each engine executes in order), so the
program is laid out as 12 "windows" (one attention group each) with other
PE work injected between kb steps as fillers:
  upfront: q-proj (first half), k-proj co0-1, v/vi c0-chunk projections
  w0-3:  + k-proj co2-5, v/vi c512 chunks     w4-6: + q-proj second half
  w7-10: + qh0 output-projection blocks       w11+tail: last ob + qh1 obs
The normalize pipeline is depth-2: group i's rowsum matmul + reciprocal
run at the end of window i; the broadcast matmul + multiplies run inside
window i+1 (between kb steps), so no PE instruction waits on a DVE chain.

PSUM (8 banks): 3 score/filler slots + 4 U slots + 1 rowsum/bcast slot.
"""

import numpy as np
import concourse.bass as bass
import concourse.tile as tile
from concourse import bacc, mybir

P = 128
NTOK = 1024
C = 768
H = 12
DH = 64
CT = C // P  # 6 c-tiles
TB = NTOK // P  # 8 token blocks
QH = 2  # q halves
KB = 8  # key blocks
NQ = 512
SCALE = DH**-0.5
F32 = mybir.dt.float32
F16 = mybir.dt.float16
EXP = mybir.ActivationFunctionType.Exp
MULT = mybir.AluOpType.mult
ADD = mybir.AluOpType.add

XNAMES = ("xq", "xk", "xv", "xvi")
WNAMES = ("wq", "wk", "wv", "wvi", "wp", "wpi")


def build_module(num_devices=8, loop_n=1, stages="123", ladder_up="dve"):
    nc = bacc.Bacc(
        "TRN2", target_bir_lowering=False, debug=False, num_devices=num_devices
    )
    d = {}
    for nm in XNAMES:
        d[nm] = nc.dram_tensor(nm, [C, NTOK], F16, kind="ExternalInput").ap()
    for nm in WNAMES:
        d[nm] = nc.dram_tensor(nm, [C, C], F16, kind="ExternalInput").ap()
    d["ones"] = nc.dram_tensor("ones", [P, P], F16, kind="ExternalInput").ap()
    d["bp"] = nc.dram_tensor("bp", [P, C], F16, kind="ExternalInput").ap()
    d["bpi"] = nc.dram_tensor("bpi", [P, C], F16, kind="ExternalInput").ap()
    xo = nc.dram_tensor("xo", [NTOK, C], F16, kind="ExternalOutput").ap()
    xio = nc.dram_tensor("xio", [NTOK, C], F16, kind="ExternalOutput").ap()

    with tile.TileContext(nc) as tc:
        with (
            tc.tile_pool(name="persist", bufs=1) as pp,
            tc.tile_pool(name="wk", bufs=4) as wk,
            tc.tile_pool(name="espool", bufs=4) as espool,
            tc.tile_pool(name="ubp", bufs=3) as ubp,
            tc.tile_pool(name="nrm", bufs=3) as nrm,
            tc.tile_pool(name="tmp", bufs=3) as tmpp,
            tc.tile_pool(name="ot", bufs=3) as otp,
            tc.tile_pool(name="psp", bufs=2, space="PSUM") as psp,
            tc.tile_pool(name="pss", bufs=1, space="PSUM") as pss,
            tc.tile_pool(name="psu", bufs=1, space="PSUM") as psu,
            tc.tile_pool(name="psn", bufs=1, space="PSUM") as psn,
        ):
            qbt = pp.tile([P, CT, NTOK], F16, tag="qbt")
            kbt = pp.tile([P, CT, NTOK], F16, tag="kbt")
            vcomb = pp.tile([P, TB, H, P], F16, tag="vcomb")
            axt = pp.tile([P, CT, NTOK], F16, tag="axt")
            ait = pp.tile([P, CT, NTOK], F16, tag="ait")
            onest = pp.tile([P, P], F16, tag="onest")
            bpr = pp.tile([P, C], F16, tag="bpr")
            bpir = pp.tile([P, C], F16, tag="bpir")
            wpt = pp.tile([P, CT, C], F16, tag="wpt")
            wpit = pp.tile([P, CT, C], F16, tag="wpit")
            xq = pp.tile([P, CT, NTOK], F16, tag="xq")
            xk = pp.tile([P, CT, NTOK], F16, tag="xk")
            xv = pp.tile([P, CT, NTOK], F16, tag="xv")
            xvi = pp.tile([P, CT, NTOK], F16, tag="xvi")
            wq = pp.tile([P, CT, C], F16, tag="wq")
            wk_ = pp.tile([P, CT, C], F16, tag="wk")
            wv_ = pp.tile([P, CT, C], F16, tag="wv")
            wvi_ = pp.tile([P, CT, C], F16, tag="wvi")

            def chunk_dma(dst, nm):
                for ci in range(CT):
                    nc.sync.dma_start(
                        dst[:, ci, :], d[nm][ci * P : (ci + 1) * P, :]
                    )

            def chain_ps(pool):
                if pool is psp:
                    t = psp.tile([P, 2, NQ], F32, tag="sp", name="chps2")
                    return t[:, 0, :]
                t = pss.tile([P, NQ], F32, tag="ps", name="chps")
                return t

            def proj_chain_t(dst, xt, wt, co, nh, pool=None):
                """One output chain of a transposed projection (W @ x^T)."""
                ps = chain_ps(pool)
                for ci in range(CT):
                    nc.tensor.matmul(
                        ps[:],
                        wt[:, ci, co * P : (co + 1) * P],
                        xt[:, ci, nh * NQ : (nh + 1) * NQ],
                        start=(ci == 0),
                        stop=(ci == CT - 1),
                    )
                nc.vector.tensor_copy(dst[:, co, nh * NQ : (nh + 1) * NQ], ps[:])

            def vvi_chain(xt, wt, tb, c0, cw, off, pool=None):
                """One (token-block, col-chunk) chain of a v/v_img projection."""
                h0, h1 = c0 // DH, (c0 + cw) // DH
                ps = chain_ps(pool)
                for ci in range(CT):
                    nc.tensor.matmul(
                        ps[:, :cw],
                        xt[:, ci, tb * P : (tb + 1) * P],
                        wt[:, ci, c0 : c0 + cw],
                        start=(ci == 0),
                        stop=(ci == CT - 1),
                    )
                nc.vector.tensor_copy(
                    vcomb[:, tb, h0:h1, off : off + DH],
                    ps[:, :cw].rearrange("p (h dh) -> p h dh", dh=DH),
                )

            def out_block(tb, stream, pools=(None, None)):
                """One (token-block, stream) of the output projection."""
                src, wt, bias_t, dst_dram = (
                    (axt, wpt, bpr, xo) if stream == 0 else (ait, wpit, bpir, xio)
                )
                ot = otp.tile([P, C], F16, tag="ot")
                for (c0, cw), pl in zip(((0, 512), (512, 256)), pools):
                    ps = chain_ps(pl)
                    for ci in range(CT):
                        nc.tensor.matmul(
                            ps[:, :cw],
                            src[:, ci, tb * P : (tb + 1) * P],
                            wt[:, ci, c0 : c0 + cw],
                            start=(ci == 0),
                            stop=(ci == CT - 1),
                        )
                    nc.vector.tensor_tensor(
                        ot[:, c0 : c0 + cw], ps[:, :cw],
                        bias_t[:, c0 : c0 + cw], ADD,
                    )
                nc.sync.dma_start(dst_dram[tb * P : (tb + 1) * P, :], ot[:])

            # ---- attention group (window) with injected fillers ----

            def norm_lo(stash):
                ct, qsl, ub_lo, ub_up, rc_l, rs_u = stash
                rp = psn.tile([P, NQ], F32, tag="rn")
                nc.tensor.matmul(
                    rp[:], onest[0:1, :], rc_l[:], start=True, stop=True
                )
                nc.vector.tensor_tensor(
                    axt[0:DH, ct, qsl], ub_lo[0:DH, :], rp[0:DH, :], MULT
                )
                t_il = tmpp.tile([P, NQ], F16, tag="tshift")
                nc.vector.tensor_tensor(
                    t_il[DH:P, :], ub_lo[DH:P, :], rp[DH:P, :], MULT
                )
                nc.sync.dma_start(ait[0:DH, ct, qsl], t_il[DH:P, :])

            def norm_up(stash):
                ct, qsl, ub_lo, ub_up, rc_l, rs_u = stash
                rp = psn.tile([P, NQ], F32, tag="rn")
                nc.tensor.matmul(
                    rp[:], onest[32:33, :], rs_u[32:33, :], start=True, stop=True
                )
                nc.vector.tensor_tensor(
                    ait[DH:P, ct, qsl], ub_up[DH:P, :], rp[DH:P, :], MULT
                )
                t_xu = tmpp.tile([P, NQ], F16, tag="tshift")
                nc.vector.tensor_tensor(
                    t_xu[0:DH, :], ub_up[0:DH, :], rp[0:DH, :], MULT
                )
                nc.sync.dma_start(axt[DH:P, ct, qsl], t_xu[0:DH, :])

            def window(ct, qh, prev, fillers):
                """One attention group. prev's normalize + fillers are
                interleaved between kb steps. Returns this group's stash."""
                h_lo, h_up = 2 * ct, 2 * ct + 1
                qsl = slice(qh * NQ, (qh + 1) * NQ)
                upair = psu.tile([P, 2, NQ], F32, tag="u")
                u_lo, u_up = upair[:, 0, :], upair[:, 1, :]
                es = []
                fq = list(fillers)
                run_l = run_u = None
                spair = None
                for kb in range(KB):
                    ksl = slice(kb * P, (kb + 1) * P)
                    # two heads of one key block share a 2-bank PSUM pair so a
                    # single exp instruction covers both
                    spair = psp.tile([P, 2, NQ], F32, tag="sp")
                    nc.tensor.matmul(
                        spair[:, 0, :], kbt[0:DH, ct, ksl], qbt[0:DH, ct, qsl],
                        start=True, stop=True,
                    )
                    nc.tensor.matmul(
                        spair[:, 1, :], kbt[DH:P, ct, ksl], qbt[DH:P, ct, qsl],
                        start=True, stop=True,
                    )
                    epair = wk.tile([P, 2, NQ], F16, tag="ep")
                    nc.scalar.activation(epair[:], spair[:], EXP, scale=SCALE)
                    e_lo, e_up = epair[:, 0, :], epair[:, 1, :]
                    es.append((e_lo, e_up))
                    if kb > 0:
                        pkb = kb - 1
                        st = pkb == 0
                        nc.tensor.matmul(
                            u_lo[:], vcomb[:, pkb, h_lo, :], es[pkb][0],
                            start=st, stop=False,
                        )
                        nc.tensor.matmul(
                            u_up[:], vcomb[:, pkb, h_up, :], es[pkb][1],
                            start=st, stop=False,
                        )
                    # esum ladder (DVE, fp16 2x mode)
                    if kb == 1:
                        run_l = espool.tile([P, NQ], F16, tag="run")
                        run_u = espool.tile([P, NQ], F16, tag="run")
                        nc.vector.tensor_tensor(
                            run_l[:], es[0][0], es[1][0], ADD
                        )
                        ueng = nc.gpsimd if ladder_up == "gpsimd" else nc.vector
                        ueng.tensor_tensor(
                            run_u[:], es[0][1], es[1][1], ADD
                        )
                    elif kb > 1:
                        nrun_l = espool.tile([P, NQ], F16, tag="run")
                        nrun_u = espool.tile([P, NQ], F16, tag="run")
                        nc.vector.tensor_tensor(
                            nrun_l[:], run_l[:], es[kb][0], ADD
                        )
                        eng = (nc.vector if (kb == KB - 1 or ladder_up != "gpsimd")
                               else nc.gpsimd)
                        eng.tensor_tensor(
                            nrun_u[:], run_u[:], es[kb][1], ADD
                        )
                        run_l, run_u = nrun_l, nrun_u
                    # injected work between kb steps
                    if kb == 2 and prev is not None:
                        norm_lo(prev)
                    if kb == 4 and prev is not None:
                        norm_up(prev)
                    if kb in (1, 3) and fq:
                        fq.pop(0)()
                nc.tensor.matmul(
                    u_lo[:], vcomb[:, KB - 1, h_lo, :], es[KB - 1][0],
                    start=False, stop=True,
                )
                nc.tensor.matmul(
                    u_up[:], vcomb[:, KB - 1, h_up, :], es[KB - 1][1],
                    start=False, stop=True,
                )
                while fq:
                    fq.pop(0)()
                # drain U to SBUF f16 (frees PSUM; DVE multiplies may only
                # read one PSUM operand)
                ubpair = ubp.tile([P, 2, NQ], F16, tag="ub")
                nc.vector.tensor_copy(ubpair[:], upair[:])
                ub_lo, ub_up = ubpair[:, 0, :], ubpair[:, 1, :]
                # rowsum (single ones-matmul per half) + reciprocal
                r2 = psn.tile([33, NQ], F32, tag="rn")
                nc.tensor.matmul(
                    r2[0:1, :], onest[:, 0:1], run_l[:], start=True, stop=True
                )
                nc.tensor.matmul(
                    r2[32:33, :], onest[:, 0:1], run_u[:], start=True, stop=True
                )
                rs = nrm.tile([33, NQ], F16, tag="rs")
                with nc.allow_low_precision(reason="softmax recip in fp16"):
                    nc.vector.reciprocal(rs[:], r2[:])
                return (ct, qsl, ub_lo, ub_up, rs[0:1, :], rs)

            def body():
                # ---- DMA issuance (in needed-first order) ----
                nc.sync.dma_start(wq[:, 0, 0:P], d["wq"][0:P, 0:P])
                nc.sync.dma_start(xq[:, 0, 0:NQ], d["xq"][0:P, 0:NQ])
                for wt_, wnm, xt_, xnm in (
                    (wq, "wq", xq, "xq"), (wk_, "wk", xk, "xk"),
                    (wv_, "wv", xv, "xv"), (wvi_, "wvi", xvi, "xvi"),
                ):
                    for ci in range(CT):
                        if wt_ is wq and ci == 0:
                            nc.sync.dma_start(
                                wt_[:, 0, P:C], d[wnm][0:P, P:C]
                            )
                            nc.sync.dma_start(
                                xt_[:, 0, NQ:NTOK], d[xnm][0:P, NQ:NTOK]
                            )
                            continue
                        nc.sync.dma_start(
                            wt_[:, ci, :], d[wnm][ci * P : (ci + 1) * P, :]
                        )
                        nc.sync.dma_start(
                            xt_[:, ci, :], d[xnm][ci * P : (ci + 1) * P, :]
                        )
                nc.sync.dma_start(onest[:], d["ones"])
                nc.sync.dma_start(bpr[:], d["bp"])
                nc.sync.dma_start(bpir[:], d["bpi"])
                chunk_dma(wpt, "wp")
                chunk_dma(wpit, "wpi")

                # ---- upfront PE work (toggle PSUM pools: scores pool is
                # idle here, so chains round-robin psp/pss for overlap) ----
                tog = [0]

                def cyc():
                    tog[0] ^= 1
                    return psp if tog[0] else pss

                for co in range(CT):
                    proj_chain_t(qbt, xq, wq, co, 0, pool=cyc())
                for co in range(CT):
                    for nh in range(QH):
                        proj_chain_t(kbt, xk, wk_, co, nh, pool=cyc())
                for tb in range(TB):
                    vvi_chain(xv, wv_, tb, 0, 512, 0, pool=cyc())
                    vvi_chain(xvi, wvi_, tb, 0, 512, DH, pool=cyc())

                # ---- filler schedules per window ----
                def vfill(tb):
                    return [
                        lambda tb=tb: vvi_chain(xv, wv_, tb, 512, 256, 0),
                        lambda tb=tb: vvi_chain(xvi, wvi_, tb, 512, 256, DH),
                    ]

                def qfill(cos):
                    return [
                        lambda co=co: proj_chain_t(qbt, xq, wq, co, 1)
                        for co in cos
                    ]

                def ofill(obs):
                    return [
                        lambda tb=tb, s=s: out_block(tb, s) for tb, s in obs
                    ]

                fillers = {
                    0: vfill(0) + vfill(1),
                    1: vfill(2) + vfill(3),
                    2: vfill(4) + vfill(5),
                    3: vfill(6) + vfill(7),
                    4: qfill((0, 1)),
                    5: qfill((2, 3)),
                    6: qfill((4, 5)),
                    7: ofill(((0, 0), (0, 1))),
                    8: ofill(((1, 0), (1, 1))),
                    9: ofill(((2, 0), (2, 1))),
                    10: [],
                    11: [],
                }

                groups = [(qh, ct) for qh in range(QH) for ct in range(CT)]
                prev = None
                for i, (qh, ct) in enumerate(groups):
                    prev = window(ct, qh, prev, fillers[i])

                # ---- tail: last normalize + remaining output blocks ----
                out_block(3, 0, pools=(cyc(), cyc()))
                norm_lo(prev)
                out_block(3, 1, pools=(cyc(), cyc()))
                norm_up(prev)
                for tb in range(4, TB):
                    out_block(tb, 0, pools=(cyc(), cyc()))
                    out_block(tb, 1, pools=(cyc(), cyc()))

            if loop_n == 1:
                body()
            else:
                with tc.For_i(0, loop_n, 1):
                    body()

    nc.compile()
    return nc


def make_in_maps(q, k, v, v_img, Wq, Wk, Wv, Wvim, Wp, bp, Wpi, bpi, n_cores=8):
    """Host-side prep: per-core transposed fp16 activations + shared fp16 weights."""
    f = np.float32
    h = np.float16
    shared = {
        "wq": np.asarray(Wq, f).T.astype(h),
        "wk": np.asarray(Wk, f).T.astype(h),
        "wv": np.asarray(Wv, f).T.astype(h),
        "wvi": np.asarray(Wvim, f).T.astype(h),
        "wp": np.asarray(Wp, f).T.astype(h),
        "wpi": np.asarray(Wpi, f).T.astype(h),
        "ones": np.ones((P, P), h),
        "bp": np.ascontiguousarray(np.broadcast_to(np.asarray(bp, f), (P, C))).astype(h),
        "bpi": np.ascontiguousarray(np.broadcast_to(np.asarray(bpi, f), (P, C))).astype(h),
    }
    q = np.asarray(q, f)
    k = np.asarray(k, f)
    v = np.asarray(v, f)
    vi = np.asarray(v_img, f)
    in_maps = []
    for b in range(n_cores):
        in_maps.append(
            {
                "xq": np.ascontiguousarray(q[:, b, :].T).astype(h),
                "xk": np.ascontiguousarray(k[:, b, :].T).astype(h),
                "xv": np.ascontiguousarray(v[:, b, :].T).astype(h),
                "xvi": np.ascontiguousarray(vi[:, b, :].T).astype(h),
                **shared,
            }
        )
    return in_maps


# ---------------------------------------------------------------------------
# Harness entry point: full inputs in, full outputs out.
# Shards batch B=8 across the 8 NeuronCores (data parallel), no collectives.
# ---------------------------------------------------------------------------

_NC_CACHE = {}


def _get_module():
    if "nc" not in _NC_CACHE:
        _NC_CACHE["nc"] = build_module(num_devices=8)
    return _NC_CACHE["nc"]


def kernel(q, k, v, v_img, Wq, Wk, Wv, Wvim, Wp, bp, Wpi, bpi):
    from concourse.bass_utils import run_bass_kernel_spmd

    B = np.asarray(q).shape[1]
    nc = _get_module()
    in_maps = make_in_maps(q, k, v, v_img, Wq, Wk, Wv, Wvim, Wp, bp, Wpi, bpi,
                           n_cores=B)
    res = run_bass_kernel_spmd(nc, in_maps, core_ids=list(range(B)), trace=False)
    x = np.stack([res.results[b]["xo"].astype(np.float32) for b in range(B)])
    x_im = np.stack([res.results[b]["xio"].astype(np.float32) for b in range(B)])
    return (x, x_im)
